# revision 13
# baseline (speedup 1.0000x reference)
"""Trainium2 Bass kernel: gated MoE residual block (two 3x3 convs, C=32).

  g  = gate * (gate > 0)                          # [B, C]
  h  = relu((conv3x3(x, w1) + b1) * g)
  h2 = relu((conv3x3(h, w2) + b2) * g)
  out = h2 + x

Sharding: data-parallel over batch. 16 images -> 8 cores x 2 images.

Device algorithm (per core, per image):
  - x arrives pre-packed (host-side numpy) in "mod-4 row-interleaved" SBUF
    layout: partition 32*(row%4)+ci, free = (row//4, col), zero halo baked
    in. A second copy arrives pre-rotated by 2 rows for the residual add.
    All device DMAs are fully contiguous (128 long descriptors each).
  - conv as full-size matmuls, K = M = 128: contraction over 4 row-slots x
    32 channels of one aligned 4-row window; output columns (q, co) hold 4
    CONSECUTIVE output rows (window rows + 1). Each output row's 3 dy-taps
    split between the aligned window (main) and the next window (wrap):
    2 matmuls per dx, 6 per 8-row PSUM block, all base-partition 0.
  - h stays on-chip with +1 row phase so conv2 reuses the same structure.
  - epilogue on ScalarE: relu(psum * g + b*g) straight from PSUM.
  - conv2 epilogue + residual add on VectorE into a full-image staging
    buffer, stored with one contiguous DMA; host de-interleaves.
"""

import numpy as np
import ml_dtypes

import concourse.bass as bass
import concourse.tile as tile
from concourse import bacc, mybir

B, C, H, W = 16, 32, 256, 256
IMGS_PER_CORE = 2
N_CORES = 8
KW = 3
S = 4            # row interleave factor (slots per window)
A = H // S       # 64 aligned 4-row windows
WP = W + 2       # padded row width (zero cols 0 and 257)
NSX = A + 3      # x_il slots: idx = window + 1; idx 0, A+1, A+2 zero
NSR = A + 2      # x_rot/out_stage slots (phase-2): idx 0..A+1
J = 2            # windows per PSUM block: N = J*W = 512
F32 = mybir.dt.float32
BF16 = mybir.dt.bfloat16
NV = 2 * KW      # weight matrices per layer: (main, wrap) x 3 dx
BLOCKS = [-1] + list(range(1, A, J))


def _pack_weights(w: np.ndarray) -> np.ndarray:
    """w: [C_out, C_in, 3, 3] (OIHW) -> [NV, 128, 128] lhsT stack.

    Block (s, q) of main[dx] = w[:, :, s-q, dx].T   (0 <= s-q <= 2)
    Block (s, q) of wrap[dx] = w[:, :, 4+s-q, dx].T (0 <= 4+s-q <= 2)
    lhsT[(32s+ci), (32q+co)]; out row (window k) = 4k+1+q.
    """
    wv = np.zeros((NV, S * C, S * C), dtype=np.float32)
    for dx in range(KW):
        for q in range(S):
            for s in range(S):
                if 0 <= s - q <= 2:
                    wv[2 * dx, 32 * s:32 * s + 32, 32 * q:32 * q + 32] = \
                        w[:, :, s - q, dx].T
                if 0 <= 4 + s - q <= 2:
                    wv[2 * dx + 1, 32 * s:32 * s + 32, 32 * q:32 * q + 32] = \
                        w[:, :, 4 + s - q, dx].T
    return wv


def _interleave_x(x: np.ndarray) -> tuple[np.ndarray, np.ndarray]:
    """x: [n, C, H, W] f32 -> (x_il [n,128,NSX,WP], x_rot [n,128,NSR,W]) bf16.

    x_il:  partition 32s+ci holds row 4(i-1)+s at slot i, col c+1 (zero halo).
    x_rot: partition 32q+ci holds row 4(i-1)+2+q at slot i (no col pad).
    """
    n = x.shape[0]
    xb = x.astype(ml_dtypes.bfloat16)

    ext = np.zeros((n, C, S * NSX, W), dtype=ml_dtypes.bfloat16)
    ext[:, :, S:S + H, :] = xb
    il = ext.reshape(n, C, NSX, S, W).transpose(0, 3, 1, 2, 4) \
            .reshape(n, S * C, NSX, W)
    x_il = np.zeros((n, S * C, NSX, WP), dtype=ml_dtypes.bfloat16)
    x_il[:, :, :, 1:1 + W] = il

    ext2 = np.zeros((n, C, S * NSR, W), dtype=ml_dtypes.bfloat16)
    ext2[:, :, 2:2 + H, :] = xb
    x_rot = ext2.reshape(n, C, NSR, S, W).transpose(0, 3, 1, 2, 4) \
               .reshape(n, S * C, NSR, W)
    return np.ascontiguousarray(x_il), np.ascontiguousarray(x_rot)


def _deinterleave_out(dev: np.ndarray) -> np.ndarray:
    """dev: [n, 128, NSR, W] (row z = 4(i-1)+2+q at partition 32q+co)
    -> [n, C, H, W] f32."""
    dev = np.asarray(dev).astype(np.float32)
    n = dev.shape[0]
    v = dev.reshape(n, S, C, NSR, W).transpose(0, 2, 3, 1, 4) \
           .reshape(n, C, S * NSR, W)
    return np.ascontiguousarray(v[:, :, 2:2 + H, :])


def _build_core_graph(reps: int = 1):
    nc = bacc.Bacc(None, target_bir_lowering=False, debug=True)

    xil_ext = nc.declare_dram_parameter("xil", [IMGS_PER_CORE, S * C, NSX, WP], BF16, isOutput=False)
    xrot_ext = nc.declare_dram_parameter("xrot", [IMGS_PER_CORE, S * C, NSR, W], BF16, isOutput=False)
    wv1_ext = nc.declare_dram_parameter("wv1", [NV, S * C, S * C], BF16, isOutput=False)
    wv2_ext = nc.declare_dram_parameter("wv2", [NV, S * C, S * C], BF16, isOutput=False)
    gv_ext = nc.declare_dram_parameter("gv", [S * C, IMGS_PER_CORE], F32, isOutput=False)
    bg1_ext = nc.declare_dram_parameter("bg1", [S * C, IMGS_PER_CORE], F32, isOutput=False)
    bg2_ext = nc.declare_dram_parameter("bg2", [S * C, IMGS_PER_CORE], F32, isOutput=False)
    out_ext = nc.declare_dram_parameter("out", [IMGS_PER_CORE, S * C, NSR, W], BF16, isOutput=True)

    with tile.TileContext(nc) as tc:
        with (
            tc.tile_pool(name="const", bufs=1) as cpool,
            tc.tile_pool(name="xb", bufs=1) as xpool,
            tc.tile_pool(name="os", bufs=2) as ospool,
            tc.tile_pool(name="hb", bufs=1) as hpool,
            tc.tile_pool(name="ps", bufs=8, space=bass.MemorySpace.PSUM) as pspool,
            tc.tile_pool(name="ep", bufs=4) as epool,
        ):
            wv1_t = cpool.tile([S * C, NV, S * C], BF16)
            wv2_t = cpool.tile([S * C, NV, S * C], BF16)
            gv_t = cpool.tile([S * C, IMGS_PER_CORE], F32)
            bg1_t = cpool.tile([S * C, IMGS_PER_CORE], F32)
            bg2_t = cpool.tile([S * C, IMGS_PER_CORE], F32)
            nc.sync.dma_start(out=wv1_t[:], in_=wv1_ext.rearrange("v p c -> p v c"))
            nc.sync.dma_start(out=wv2_t[:], in_=wv2_ext.rearrange("v p c -> p v c"))
            nc.sync.dma_start(out=gv_t[:], in_=gv_ext[:])
            nc.sync.dma_start(out=bg1_t[:], in_=bg1_ext[:])
            nc.sync.dma_start(out=bg2_t[:], in_=bg2_ext[:])

            for img in [i for _ in range(reps) for i in range(IMGS_PER_CORE)]:
                x_il = xpool.tile([S * C, NSX, WP], BF16)
                x_rot = xpool.tile([S * C, NSR, W], BF16, tag="x_rot")
                out_stage = ospool.tile([S * C, NSR, W], BF16)
                h_il = hpool.tile([S * C, NSX, WP], BF16)

                for c0 in range(0, NSX, 17):
                    c1 = min(c0 + 17, NSX)
                    nc.sync.dma_start(out=x_il[:, c0:c1, :],
                                      in_=xil_ext[img, :, c0:c1, :])
                for c0 in range(0, NSR, 17):
                    c1 = min(c0 + 17, NSR)
                    nc.sync.dma_start(out=x_rot[:, c0:c1, :],
                                      in_=xrot_ext[img, :, c0:c1, :])

                # h halo: zero slots 0, A+1, A+2 and cols 0, WP-1
                nc.vector.memset(h_il[:, 0, :], 0.0)
                nc.vector.memset(h_il[:, A + 1, :], 0.0)
                nc.vector.memset(h_il[:, A + 2, :], 0.0)
                nc.vector.memset(h_il[:, :, 0], 0.0)
                nc.vector.memset(h_il[:, :, WP - 1], 0.0)

                def conv_blocks(src, wv_t):
                    for k0 in BLOCKS:
                        ps = pspool.tile([S * C, J, W], F32)
                        for dx in range(KW):
                            for wi, da in ((0, 0), (1, 1)):  # main, wrap
                                lo = k0 + 1 + da
                                nc.tensor.matmul(
                                    ps[:, :, :],
                                    wv_t[:, 2 * dx + wi, :],
                                    src[:, lo:lo + J, dx:dx + W],
                                    start=(dx == 0 and wi == 0),
                                    stop=(dx == KW - 1 and wi == 1),
                                    skip_group_check=True,
                                )
                        yield k0, ps

                # ---- conv1: x_il -> h_il (h stored with +1 row phase) ----
                for k0, ps in conv_blocks(x_il, wv1_t):
                    nc.scalar.activation(
                        h_il[:, k0 + 1:k0 + 1 + J, 1:1 + W],
                        ps[:, :, :],
                        mybir.ActivationFunctionType.Relu,
                        bias=bg1_t[:, img:img + 1],
                        scale=gv_t[:, img:img + 1],
                    )

                # re-zero h pad slots that got edge-garbage from conv1
                nc.vector.memset(h_il[0:3 * C, 0, :], 0.0)
                nc.vector.memset(h_il[3 * C:4 * C, A, :], 0.0)
                nc.vector.memset(h_il[:, A + 1, :], 0.0)

                # ---- conv2 + residual into out_stage ----
                for m0, ps in conv_blocks(h_il, wv2_t):
                    tt = epool.tile([S * C, J, W], BF16, tag="tt")
                    nc.scalar.activation(
                        tt[:], ps[:, :, :],
                        mybir.ActivationFunctionType.Relu,
                        bias=bg2_t[:, img:img + 1],
                        scale=gv_t[:, img:img + 1],
                    )
                    # out row z = 4(m0+j)+2+q lives at idx m0+j+1; x_rot pads
                    # are zero and edge garbage lands in out_stage pad slots
                    nc.vector.tensor_tensor(
                        out_stage[:, m0 + 1:m0 + 1 + J, :], tt[:],
                        x_rot[:, m0 + 1:m0 + 1 + J, :],
                        mybir.AluOpType.add,
                    )

                nc.gpsimd.dma_start(out=out_ext[img], in_=out_stage[:])

    nc.compile()
    return nc


def _host_prep(x, gate_values, w1, b1, w2, b2):
    x = np.ascontiguousarray(np.asarray(x, dtype=np.float32))
    gate_values = np.asarray(gate_values, dtype=np.float32)
    w1 = np.asarray(w1, dtype=np.float32)
    b1 = np.asarray(b1, dtype=np.float32)
    w2 = np.asarray(w2, dtype=np.float32)
    b2 = np.asarray(b2, dtype=np.float32)

    g = gate_values * (gate_values > 0)                      # [B, C]
    wv1 = _pack_weights(w1).astype(ml_dtypes.bfloat16)
    wv2 = _pack_weights(w2).astype(ml_dtypes.bfloat16)

    in_maps = []
    for core in range(N_CORES):
        sl = slice(core * IMGS_PER_CORE, (core + 1) * IMGS_PER_CORE)
        gc = g[sl]                                           # [2, C]
        x_il, x_rot = _interleave_x(x[sl])
        in_maps.append({
            "xil": x_il, "xrot": x_rot,
            "wv1": wv1, "wv2": wv2,
            "gv": np.ascontiguousarray(np.tile(gc.T, (S, 1))),
            "bg1": np.ascontiguousarray(np.tile((gc * b1[None, :]).T, (S, 1))),
            "bg2": np.ascontiguousarray(np.tile((gc * b2[None, :]).T, (S, 1))),
        })
    return in_maps


_NC_CACHE = None


def _get_graph():
    global _NC_CACHE
    if _NC_CACHE is None:
        _NC_CACHE = _build_core_graph()
    return _NC_CACHE


def kernel(x, gate_values, w1, b1, w2, b2, _trace=False, **_ignored):
    from concourse.bass_utils import run_bass_kernel_spmd

    nc = _get_graph()
    in_maps = _host_prep(x, gate_values, w1, b1, w2, b2)
    res = run_bass_kernel_spmd(
        nc, in_maps, core_ids=list(range(N_CORES)), trace=_trace)
    outs = [_deinterleave_out(res.results[i]["out"]) for i in range(N_CORES)]
    full = np.concatenate(outs, axis=0).astype(np.float32)
    if _trace:
        return full, res
    return full


# revision 14
# speedup vs baseline: 7.3913x; 7.3913x over previous
"""Trainium2 Bass kernel: gated MoE residual block (two 3x3 convs, C=32).

  g  = gate * (gate > 0)                          # [B, C]
  h  = relu((conv3x3(x, w1) + b1) * g)
  h2 = relu((conv3x3(h, w2) + b2) * g)
  out = h2 + x

Sharding: data-parallel over batch. 16 images -> 8 cores x 2 images.

Device algorithm (per core, per image):
  - x arrives pre-packed (host-side numpy) in "mod-4 row-interleaved" SBUF
    layout: partition 32*(row%4)+ci, free = (row//4, col), zero halo baked
    in. A second copy arrives pre-rotated by 2 rows for the residual add.
    All device DMAs are fully contiguous (128 long descriptors each).
  - conv as full-size matmuls, K = M = 128: contraction over 4 row-slots x
    32 channels of one aligned 4-row window; output columns (q, co) hold 4
    CONSECUTIVE output rows (window rows + 1). Each output row's 3 dy-taps
    split between the aligned window (main) and the next window (wrap):
    2 matmuls per dx, 6 per 8-row PSUM block, all base-partition 0.
  - h stays on-chip with +1 row phase so conv2 reuses the same structure.
  - epilogue on ScalarE: relu(psum * g + b*g) straight from PSUM.
  - conv2 epilogue + residual add on VectorE into a full-image staging
    buffer, stored with one contiguous DMA; host de-interleaves.
"""

import numpy as np
import ml_dtypes

import concourse.bass as bass
import concourse.tile as tile
from concourse import bacc, mybir

B, C, H, W = 16, 32, 256, 256
IMGS_PER_CORE = 2
N_CORES = 8
KW = 3
S = 4            # row interleave factor (slots per window)
A = H // S       # 64 aligned 4-row windows
WP = W + 2       # padded row width (zero cols 0 and 257)
NSX = A + 3      # x_il slots: idx = window + 1; idx 0, A+1, A+2 zero
NSR = A + 2      # x_rot/out_stage slots (phase-2): idx 0..A+1
J = 2            # windows per PSUM block: N = J*W = 512
F32 = mybir.dt.float32
BF16 = mybir.dt.bfloat16
NV = 2 * KW      # weight matrices per layer: (main, wrap) x 3 dx
BLOCKS = [-1] + list(range(1, A, J))


def _pack_weights(w: np.ndarray) -> np.ndarray:
    """w: [C_out, C_in, 3, 3] (OIHW) -> [NV, 128, 128] lhsT stack.

    Block (s, q) of main[dx] = w[:, :, s-q, dx].T   (0 <= s-q <= 2)
    Block (s, q) of wrap[dx] = w[:, :, 4+s-q, dx].T (0 <= 4+s-q <= 2)
    lhsT[(32s+ci), (32q+co)]; out row (window k) = 4k+1+q.
    """
    wv = np.zeros((NV, S * C, S * C), dtype=np.float32)
    for dx in range(KW):
        for q in range(S):
            for s in range(S):
                if 0 <= s - q <= 2:
                    wv[2 * dx, 32 * s:32 * s + 32, 32 * q:32 * q + 32] = \
                        w[:, :, s - q, dx].T
                if 0 <= 4 + s - q <= 2:
                    wv[2 * dx + 1, 32 * s:32 * s + 32, 32 * q:32 * q + 32] = \
                        w[:, :, 4 + s - q, dx].T
    return wv


def _interleave_x(x: np.ndarray) -> tuple[np.ndarray, np.ndarray]:
    """x: [n, C, H, W] f32 -> (x_il [n,128,NSX,WP], x_rot [n,128,NSR,W]) bf16.

    x_il:  partition 32s+ci holds row 4(i-1)+s at slot i, col c+1 (zero halo).
    x_rot: partition 32q+ci holds row 4(i-1)+2+q at slot i (no col pad).
    """
    n = x.shape[0]
    xb = x.astype(ml_dtypes.bfloat16)

    ext = np.zeros((n, C, S * NSX, W), dtype=ml_dtypes.bfloat16)
    ext[:, :, S:S + H, :] = xb
    il = ext.reshape(n, C, NSX, S, W).transpose(0, 3, 1, 2, 4) \
            .reshape(n, S * C, NSX, W)
    x_il = np.zeros((n, S * C, NSX, WP), dtype=ml_dtypes.bfloat16)
    x_il[:, :, :, 1:1 + W] = il

    ext2 = np.zeros((n, C, S * NSR, W), dtype=ml_dtypes.bfloat16)
    ext2[:, :, 2:2 + H, :] = xb
    x_rot = ext2.reshape(n, C, NSR, S, W).transpose(0, 3, 1, 2, 4) \
               .reshape(n, S * C, NSR, W)
    return np.ascontiguousarray(x_il), np.ascontiguousarray(x_rot)


def _deinterleave_out(dev: np.ndarray) -> np.ndarray:
    """dev: [n, 128, NSR, W] (row z = 4(i-1)+2+q at partition 32q+co)
    -> [n, C, H, W] f32."""
    dev = np.asarray(dev).astype(np.float32)
    n = dev.shape[0]
    v = dev.reshape(n, S, C, NSR, W).transpose(0, 2, 3, 1, 4) \
           .reshape(n, C, S * NSR, W)
    return np.ascontiguousarray(v[:, :, 2:2 + H, :])


def _build_core_graph(reps: int = 1):
    nc = bacc.Bacc(None, target_bir_lowering=False, debug=False)

    xil_ext = nc.declare_dram_parameter("xil", [IMGS_PER_CORE, S * C, NSX, WP], BF16, isOutput=False)
    xrot_ext = nc.declare_dram_parameter("xrot", [IMGS_PER_CORE, S * C, NSR, W], BF16, isOutput=False)
    wv1_ext = nc.declare_dram_parameter("wv1", [NV, S * C, S * C], BF16, isOutput=False)
    wv2_ext = nc.declare_dram_parameter("wv2", [NV, S * C, S * C], BF16, isOutput=False)
    gv_ext = nc.declare_dram_parameter("gv", [S * C, IMGS_PER_CORE], F32, isOutput=False)
    bg1_ext = nc.declare_dram_parameter("bg1", [S * C, IMGS_PER_CORE], F32, isOutput=False)
    bg2_ext = nc.declare_dram_parameter("bg2", [S * C, IMGS_PER_CORE], F32, isOutput=False)
    out_ext = nc.declare_dram_parameter("out", [IMGS_PER_CORE, S * C, NSR, W], BF16, isOutput=True)

    with tile.TileContext(nc) as tc:
        with (
            tc.tile_pool(name="const", bufs=1) as cpool,
            tc.tile_pool(name="xb", bufs=1) as xpool,
            tc.tile_pool(name="os", bufs=2) as ospool,
            tc.tile_pool(name="hb", bufs=1) as hpool,
            tc.tile_pool(name="ps", bufs=8, space=bass.MemorySpace.PSUM) as pspool,
            tc.tile_pool(name="ep", bufs=4) as epool,
        ):
            wv1_t = cpool.tile([S * C, NV, S * C], BF16)
            wv2_t = cpool.tile([S * C, NV, S * C], BF16)
            gv_t = cpool.tile([S * C, IMGS_PER_CORE], F32)
            bg1_t = cpool.tile([S * C, IMGS_PER_CORE], F32)
            bg2_t = cpool.tile([S * C, IMGS_PER_CORE], F32)
            nc.sync.dma_start(out=wv1_t[:], in_=wv1_ext.rearrange("v p c -> p v c"))
            nc.sync.dma_start(out=wv2_t[:], in_=wv2_ext.rearrange("v p c -> p v c"))
            nc.sync.dma_start(out=gv_t[:], in_=gv_ext[:])
            nc.sync.dma_start(out=bg1_t[:], in_=bg1_ext[:])
            nc.sync.dma_start(out=bg2_t[:], in_=bg2_ext[:])

            for img in [i for _ in range(reps) for i in range(IMGS_PER_CORE)]:
                x_il = xpool.tile([S * C, NSX, WP], BF16)
                x_rot = xpool.tile([S * C, NSR, W], BF16, tag="x_rot")
                out_stage = ospool.tile([S * C, NSR, W], BF16)
                h_il = hpool.tile([S * C, NSX, WP], BF16)

                for c0 in range(0, NSX, 17):
                    c1 = min(c0 + 17, NSX)
                    nc.sync.dma_start(out=x_il[:, c0:c1, :],
                                      in_=xil_ext[img, :, c0:c1, :])
                for c0 in range(0, NSR, 17):
                    c1 = min(c0 + 17, NSR)
                    nc.sync.dma_start(out=x_rot[:, c0:c1, :],
                                      in_=xrot_ext[img, :, c0:c1, :])

                # h halo: zero slots 0, A+1, A+2 and cols 0, WP-1
                nc.vector.memset(h_il[:, 0, :], 0.0)
                nc.vector.memset(h_il[:, A + 1, :], 0.0)
                nc.vector.memset(h_il[:, A + 2, :], 0.0)
                nc.vector.memset(h_il[:, :, 0], 0.0)
                nc.vector.memset(h_il[:, :, WP - 1], 0.0)

                def conv_blocks(src, wv_t):
                    for k0 in BLOCKS:
                        ps = pspool.tile([S * C, J, W], F32)
                        for dx in range(KW):
                            for wi, da in ((0, 0), (1, 1)):  # main, wrap
                                lo = k0 + 1 + da
                                nc.tensor.matmul(
                                    ps[:, :, :],
                                    wv_t[:, 2 * dx + wi, :],
                                    src[:, lo:lo + J, dx:dx + W],
                                    start=(dx == 0 and wi == 0),
                                    stop=(dx == KW - 1 and wi == 1),
                                    skip_group_check=True,
                                )
                        yield k0, ps

                # ---- conv1: x_il -> h_il (h stored with +1 row phase) ----
                for k0, ps in conv_blocks(x_il, wv1_t):
                    nc.scalar.activation(
                        h_il[:, k0 + 1:k0 + 1 + J, 1:1 + W],
                        ps[:, :, :],
                        mybir.ActivationFunctionType.Relu,
                        bias=bg1_t[:, img:img + 1],
                        scale=gv_t[:, img:img + 1],
                    )

                # re-zero h pad slots that got edge-garbage from conv1
                nc.vector.memset(h_il[0:3 * C, 0, :], 0.0)
                nc.vector.memset(h_il[3 * C:4 * C, A, :], 0.0)
                nc.vector.memset(h_il[:, A + 1, :], 0.0)

                # ---- conv2 + residual into out_stage ----
                for m0, ps in conv_blocks(h_il, wv2_t):
                    tt = epool.tile([S * C, J, W], BF16, tag="tt")
                    nc.scalar.activation(
                        tt[:], ps[:, :, :],
                        mybir.ActivationFunctionType.Relu,
                        bias=bg2_t[:, img:img + 1],
                        scale=gv_t[:, img:img + 1],
                    )
                    # out row z = 4(m0+j)+2+q lives at idx m0+j+1; x_rot pads
                    # are zero and edge garbage lands in out_stage pad slots
                    nc.vector.tensor_tensor(
                        out_stage[:, m0 + 1:m0 + 1 + J, :], tt[:],
                        x_rot[:, m0 + 1:m0 + 1 + J, :],
                        mybir.AluOpType.add,
                    )

                nc.gpsimd.dma_start(out=out_ext[img], in_=out_stage[:])

    nc.compile()
    return nc


def _host_prep(x, gate_values, w1, b1, w2, b2):
    x = np.ascontiguousarray(np.asarray(x, dtype=np.float32))
    gate_values = np.asarray(gate_values, dtype=np.float32)
    w1 = np.asarray(w1, dtype=np.float32)
    b1 = np.asarray(b1, dtype=np.float32)
    w2 = np.asarray(w2, dtype=np.float32)
    b2 = np.asarray(b2, dtype=np.float32)

    g = gate_values * (gate_values > 0)                      # [B, C]
    wv1 = _pack_weights(w1).astype(ml_dtypes.bfloat16)
    wv2 = _pack_weights(w2).astype(ml_dtypes.bfloat16)

    in_maps = []
    for core in range(N_CORES):
        sl = slice(core * IMGS_PER_CORE, (core + 1) * IMGS_PER_CORE)
        gc = g[sl]                                           # [2, C]
        x_il, x_rot = _interleave_x(x[sl])
        in_maps.append({
            "xil": x_il, "xrot": x_rot,
            "wv1": wv1, "wv2": wv2,
            "gv": np.ascontiguousarray(np.tile(gc.T, (S, 1))),
            "bg1": np.ascontiguousarray(np.tile((gc * b1[None, :]).T, (S, 1))),
            "bg2": np.ascontiguousarray(np.tile((gc * b2[None, :]).T, (S, 1))),
        })
    return in_maps


_NC_CACHE = None


def _get_graph():
    global _NC_CACHE
    if _NC_CACHE is None:
        _NC_CACHE = _build_core_graph()
    return _NC_CACHE


def kernel(x, gate_values, w1, b1, w2, b2, _trace=False, **_ignored):
    from concourse.bass_utils import run_bass_kernel_spmd

    nc = _get_graph()
    in_maps = _host_prep(x, gate_values, w1, b1, w2, b2)
    res = run_bass_kernel_spmd(
        nc, in_maps, core_ids=list(range(N_CORES)), trace=_trace)
    outs = [_deinterleave_out(res.results[i]["out"]) for i in range(N_CORES)]
    full = np.concatenate(outs, axis=0).astype(np.float32)
    if _trace:
        return full, res
    return full


# revision 17
# speedup vs baseline: 1338.4377x; 181.0821x over previous
"""Trainium2 Bass kernel: gated MoE residual block (two 3x3 convs, C=32).

  g  = gate * (gate > 0)                          # [B, C]
  h  = relu((conv3x3(x, w1) + b1) * g)
  h2 = relu((conv3x3(h, w2) + b2) * g)
  out = h2 + x

Sharding: data-parallel over batch. 16 images -> 8 cores x 2 images.

Device algorithm (per core, per image):
  - x arrives pre-packed (host-side numpy) in "mod-4 row-interleaved" SBUF
    layout: partition 32*(row%4)+ci, free = (row//4, col), zero halo baked
    in. A second copy arrives pre-rotated by 2 rows for the residual add.
    All device DMAs are fully contiguous (128 long descriptors each).
  - conv as full-size matmuls, K = M = 128: contraction over 4 row-slots x
    32 channels of one aligned 4-row window; output columns (q, co) hold 4
    CONSECUTIVE output rows (window rows + 1). Each output row's 3 dy-taps
    split between the aligned window (main) and the next window (wrap):
    2 matmuls per dx, 6 per 8-row PSUM block, all base-partition 0.
  - h stays on-chip with +1 row phase so conv2 reuses the same structure.
  - epilogue on ScalarE: relu(psum * g + b*g) straight from PSUM.
  - conv2 epilogue + residual add on VectorE into a full-image staging
    buffer, stored with one contiguous DMA; host de-interleaves.
"""

import numpy as np
import ml_dtypes

import concourse.bass as bass
import concourse.tile as tile
from concourse import bacc, mybir

B, C, H, W = 16, 32, 256, 256
IMGS_PER_CORE = 2
N_CORES = 8
KW = 3
S = 4            # row interleave factor (slots per window)
A = H // S       # 64 aligned 4-row windows
WP = W + 2       # padded row width (zero cols 0 and 257)
NSX = A + 3      # x_il slots: idx = window + 1; idx 0, A+1, A+2 zero
NSR = A + 2      # x_rot/out_stage slots (phase-2): idx 0..A+1
J = 2            # windows per PSUM block: N = J*W = 512
F32 = mybir.dt.float32
BF16 = mybir.dt.bfloat16
NV = 2 * KW      # weight matrices per layer: (main, wrap) x 3 dx
BLOCKS = [-1] + list(range(1, A, J))


def _pack_weights(w: np.ndarray) -> np.ndarray:
    """w: [C_out, C_in, 3, 3] (OIHW) -> [NV, 128, 128] lhsT stack.

    Block (s, q) of main[dx] = w[:, :, s-q, dx].T   (0 <= s-q <= 2)
    Block (s, q) of wrap[dx] = w[:, :, 4+s-q, dx].T (0 <= 4+s-q <= 2)
    lhsT[(32s+ci), (32q+co)]; out row (window k) = 4k+1+q.
    """
    wv = np.zeros((NV, S * C, S * C), dtype=np.float32)
    for dx in range(KW):
        for q in range(S):
            for s in range(S):
                if 0 <= s - q <= 2:
                    wv[2 * dx, 32 * s:32 * s + 32, 32 * q:32 * q + 32] = \
                        w[:, :, s - q, dx].T
                if 0 <= 4 + s - q <= 2:
                    wv[2 * dx + 1, 32 * s:32 * s + 32, 32 * q:32 * q + 32] = \
                        w[:, :, 4 + s - q, dx].T
    return wv


def _interleave_x(x: np.ndarray) -> tuple[np.ndarray, np.ndarray]:
    """x: [n, C, H, W] f32 -> (x_il [n,128,NSX,WP], x_rot [n,128,NSR,W]) bf16.

    x_il:  partition 32s+ci holds row 4(i-1)+s at slot i, col c+1 (zero halo).
    x_rot: partition 32q+ci holds row 4(i-1)+2+q at slot i (no col pad).
    """
    n = x.shape[0]
    xb = x.astype(ml_dtypes.bfloat16)

    ext = np.zeros((n, C, S * NSX, W), dtype=ml_dtypes.bfloat16)
    ext[:, :, S:S + H, :] = xb
    il = ext.reshape(n, C, NSX, S, W).transpose(0, 3, 1, 2, 4) \
            .reshape(n, S * C, NSX, W)
    x_il = np.zeros((n, S * C, NSX, WP), dtype=ml_dtypes.bfloat16)
    x_il[:, :, :, 1:1 + W] = il

    ext2 = np.zeros((n, C, S * NSR, W), dtype=ml_dtypes.bfloat16)
    ext2[:, :, 2:2 + H, :] = xb
    x_rot = ext2.reshape(n, C, NSR, S, W).transpose(0, 3, 1, 2, 4) \
               .reshape(n, S * C, NSR, W)
    return np.ascontiguousarray(x_il), np.ascontiguousarray(x_rot)


def _deinterleave_out(dev: np.ndarray) -> np.ndarray:
    """dev: [n, 128, NSR, W] (row z = 4(i-1)+2+q at partition 32q+co)
    -> [n, C, H, W] f32."""
    dev = np.asarray(dev).astype(np.float32)
    n = dev.shape[0]
    v = dev.reshape(n, S, C, NSR, W).transpose(0, 2, 3, 1, 4) \
           .reshape(n, C, S * NSR, W)
    return np.ascontiguousarray(v[:, :, 2:2 + H, :])


def _build_core_graph(reps: int = 1):
    nc = bacc.Bacc(None, target_bir_lowering=False, debug=False)

    xil_ext = nc.declare_dram_parameter("xil", [IMGS_PER_CORE, S * C, NSX, WP], BF16, isOutput=False)
    xrot_ext = nc.declare_dram_parameter("xrot", [IMGS_PER_CORE, S * C, NSR, W], BF16, isOutput=False)
    wv1_ext = nc.declare_dram_parameter("wv1", [NV, S * C, S * C], BF16, isOutput=False)
    wv2_ext = nc.declare_dram_parameter("wv2", [NV, S * C, S * C], BF16, isOutput=False)
    gv_ext = nc.declare_dram_parameter("gv", [S * C, IMGS_PER_CORE], F32, isOutput=False)
    bg1_ext = nc.declare_dram_parameter("bg1", [S * C, IMGS_PER_CORE], F32, isOutput=False)
    bg2_ext = nc.declare_dram_parameter("bg2", [S * C, IMGS_PER_CORE], F32, isOutput=False)
    out_ext = nc.declare_dram_parameter("out", [IMGS_PER_CORE, S * C, NSR, W], BF16, isOutput=True)

    with tile.TileContext(nc) as tc:
        with (
            tc.tile_pool(name="const", bufs=1) as cpool,
            tc.tile_pool(name="xb", bufs=1) as xpool,
            tc.tile_pool(name="os", bufs=1) as ospool,
            tc.tile_pool(name="xr2", bufs=2) as xrpool,
            tc.tile_pool(name="hb", bufs=1) as hpool,
            tc.tile_pool(name="ps", bufs=8, space=bass.MemorySpace.PSUM) as pspool,
            tc.tile_pool(name="ep", bufs=4) as epool,
        ):
            wv1_t = cpool.tile([S * C, NV, S * C], BF16)
            wv2_t = cpool.tile([S * C, NV, S * C], BF16)
            gv_t = cpool.tile([S * C, IMGS_PER_CORE], F32)
            bg1_t = cpool.tile([S * C, IMGS_PER_CORE], F32)
            bg2_t = cpool.tile([S * C, IMGS_PER_CORE], F32)
            nc.sync.dma_start(out=wv1_t[:], in_=wv1_ext.rearrange("v p c -> p v c"))
            nc.sync.dma_start(out=wv2_t[:], in_=wv2_ext.rearrange("v p c -> p v c"))
            nc.sync.dma_start(out=gv_t[:], in_=gv_ext[:])
            nc.sync.dma_start(out=bg1_t[:], in_=bg1_ext[:])
            nc.sync.dma_start(out=bg2_t[:], in_=bg2_ext[:])

            for img in [i for _ in range(reps) for i in range(IMGS_PER_CORE)]:
                x_il = xpool.tile([S * C, NSX, WP], BF16)
                x_rot = xrpool.tile([S * C, NSR, W], BF16)
                out_stage = ospool.tile([S * C, NSR, W], BF16)
                h_il = hpool.tile([S * C, NSX, WP], BF16)

                xsplits = [0, 3, 11, 19, 27, 35, 43, 51, 59, NSX]
                for c0, c1 in zip(xsplits[:-1], xsplits[1:]):
                    nc.sync.dma_start(out=x_il[:, c0:c1, :],
                                      in_=xil_ext[img, :, c0:c1, :])
                for c0 in range(0, NSR, 9):
                    c1 = min(c0 + 9, NSR)
                    nc.sync.dma_start(out=x_rot[:, c0:c1, :],
                                      in_=xrot_ext[img, :, c0:c1, :])

                # h halo: zero slots 0, A+1, A+2 and cols 0, WP-1
                nc.vector.memset(h_il[:, 0, :], 0.0)
                nc.vector.memset(h_il[:, A + 1, :], 0.0)
                nc.vector.memset(h_il[:, A + 2, :], 0.0)
                nc.vector.memset(h_il[:, :, 0], 0.0)
                nc.vector.memset(h_il[:, :, WP - 1], 0.0)

                def conv_blocks(src, wv_t):
                    for k0 in BLOCKS:
                        ps = pspool.tile([S * C, J, W], F32)
                        # at the last block the wrap windows are all padding
                        pairs = [(0, 0), (1, 1)] if k0 != A - 1 else [(0, 0)]
                        mms = [(dx, wi, da) for dx in range(KW)
                               for wi, da in pairs]
                        for n, (dx, wi, da) in enumerate(mms):
                            lo = k0 + 1 + da
                            nc.tensor.matmul(
                                ps[:, :, :],
                                wv_t[:, 2 * dx + wi, :],
                                src[:, lo:lo + J, dx:dx + W],
                                start=(n == 0),
                                stop=(n == len(mms) - 1),
                                skip_group_check=True,
                            )
                        yield k0, ps

                # ---- conv1: x_il -> h_il (h stored with +1 row phase) ----
                for k0, ps in conv_blocks(x_il, wv1_t):
                    nc.scalar.activation(
                        h_il[:, k0 + 1:k0 + 1 + J, 1:1 + W],
                        ps[:, :, :],
                        mybir.ActivationFunctionType.Relu,
                        bias=bg1_t[:, img:img + 1],
                        scale=gv_t[:, img:img + 1],
                    )

                # re-zero h pad slots that got edge-garbage from conv1
                nc.vector.memset(h_il[0:3 * C, 0, :], 0.0)
                nc.vector.memset(h_il[3 * C:4 * C, A, :], 0.0)
                nc.vector.memset(h_il[:, A + 1, :], 0.0)

                # ---- conv2 + residual into out_stage ----
                for m0, ps in conv_blocks(h_il, wv2_t):
                    tt = epool.tile([S * C, J, W], BF16, tag="tt")
                    nc.scalar.activation(
                        tt[:], ps[:, :, :],
                        mybir.ActivationFunctionType.Relu,
                        bias=bg2_t[:, img:img + 1],
                        scale=gv_t[:, img:img + 1],
                    )
                    # out row z = 4(m0+j)+2+q lives at idx m0+j+1; x_rot pads
                    # are zero and edge garbage lands in out_stage pad slots
                    nc.vector.tensor_tensor(
                        out_stage[:, m0 + 1:m0 + 1 + J, :], tt[:],
                        x_rot[:, m0 + 1:m0 + 1 + J, :],
                        mybir.AluOpType.add,
                    )
                    # store completed slot ranges (8-slot chunks + final)
                    hi = m0 + 1 + J
                    if hi % 8 == 0:
                        nc.gpsimd.dma_start(
                            out=out_ext[img, :, hi - 8:hi, :],
                            in_=out_stage[:, hi - 8:hi, :])
                    elif m0 == BLOCKS[-1]:
                        lo = (hi // 8) * 8
                        nc.gpsimd.dma_start(
                            out=out_ext[img, :, lo:, :],
                            in_=out_stage[:, lo:, :])

                # (chunked stores emitted inside the conv2 loop above)

    nc.compile()
    return nc


def _host_prep(x, gate_values, w1, b1, w2, b2):
    x = np.ascontiguousarray(np.asarray(x, dtype=np.float32))
    gate_values = np.asarray(gate_values, dtype=np.float32)
    w1 = np.asarray(w1, dtype=np.float32)
    b1 = np.asarray(b1, dtype=np.float32)
    w2 = np.asarray(w2, dtype=np.float32)
    b2 = np.asarray(b2, dtype=np.float32)

    g = gate_values * (gate_values > 0)                      # [B, C]
    wv1 = _pack_weights(w1).astype(ml_dtypes.bfloat16)
    wv2 = _pack_weights(w2).astype(ml_dtypes.bfloat16)

    in_maps = []
    for core in range(N_CORES):
        sl = slice(core * IMGS_PER_CORE, (core + 1) * IMGS_PER_CORE)
        gc = g[sl]                                           # [2, C]
        x_il, x_rot = _interleave_x(x[sl])
        in_maps.append({
            "xil": x_il, "xrot": x_rot,
            "wv1": wv1, "wv2": wv2,
            "gv": np.ascontiguousarray(np.tile(gc.T, (S, 1))),
            "bg1": np.ascontiguousarray(np.tile((gc * b1[None, :]).T, (S, 1))),
            "bg2": np.ascontiguousarray(np.tile((gc * b2[None, :]).T, (S, 1))),
        })
    return in_maps


_NC_CACHE = None


def _get_graph():
    global _NC_CACHE
    if _NC_CACHE is None:
        _NC_CACHE = _build_core_graph()
    return _NC_CACHE


def kernel(x, gate_values, w1, b1, w2, b2, _trace=False, **_ignored):
    from concourse.bass_utils import run_bass_kernel_spmd

    nc = _get_graph()
    in_maps = _host_prep(x, gate_values, w1, b1, w2, b2)
    res = run_bass_kernel_spmd(
        nc, in_maps, core_ids=list(range(N_CORES)), trace=_trace)
    outs = [_deinterleave_out(res.results[i]["out"]) for i in range(N_CORES)]
    full = np.concatenate(outs, axis=0).astype(np.float32)
    if _trace:
        return full, res
    return full
